# revision 1
# baseline (speedup 1.0000x reference)
"""GP prediction kernel for Trainium2 (8 NeuronCores, data-parallel over batch).

Computes z_pred[b, p, d] = sum_c k_mult[p, c] * z_enc[b, c, d] where k_mult
is the [64, 448] GP weight matrix k_pred.T @ inv(cov + sigma*I). k_mult
depends only on compile-time constants, so it is precomputed on host; the
device work is a batched [64,448] @ [448,1024] matmul, sharded 8 batches
per core.

The fp32 operands are split on host into fp16 hi+lo pairs (z = zh + zl,
k = kh + kl) and the product is computed as kh*zh + kh*zl + kl*zh with fp32
PSUM accumulation. This keeps ~fp32 accuracy (error ~1e-6) while running
the PE at 1 cycle/row instead of fp32's 4, and moves the same number of
HBM bytes as fp32 operands would.
"""
import numpy as np
from contextlib import ExitStack

import concourse.bacc as bacc
import concourse.tile as tile
from concourse import mybir
from concourse.bass_utils import run_bass_kernel_spmd

# Problem constants (hardcoded per harness contract).
B, T, D = 64, 512, 1024
P = 64                 # N_PREDICTORS
C = T - P              # 448 context timesteps
L, SIGMA, TIMESCALE = 0.01, 0.01, 0.3
N_CORES = 8
BPC = B // N_CORES     # batches per core

KJ = [128, 128, 128, 64]          # K-tile sizes along the contraction dim
KOFF = [0, 128, 256, 384]


def _k_mult_T() -> np.ndarray:
    """[C, P] transpose of the GP weight matrix.

    Replicates the reference's fp32 jax ops on CPU so the constant matches
    the reference's k_mult near-bitwise; falls back to a float64 numpy solve.
    """
    try:
        import jax
        import jax.numpy as jnp

        cpu = jax.devices("cpu")[0]
        with jax.default_device(cpu):
            t = jnp.linspace(0.0, 1.0, T)
            t_in = t[:C] * TIMESCALE
            t_pred = t[C:] * TIMESCALE

            def rbf(x, y):
                d = x[:, None] - y[None, :]
                return jnp.exp(-0.5 * (d * d) / L)

            cov = rbf(t_in, t_in)
            k_pred = rbf(t_in, t_pred)
            eye = jnp.eye(C, dtype=cov.dtype)
            k_mult = k_pred.T @ jnp.linalg.inv(cov + eye * SIGMA)   # [P, C]
            km_T = np.asarray(k_mult).T                             # [C, P]
    except Exception:
        t = np.linspace(0.0, 1.0, T)
        t_in = t[:C] * TIMESCALE
        t_pred = t[C:] * TIMESCALE

        def rbf_np(x, y):
            d = x[:, None] - y[None, :]
            return np.exp(-0.5 * d * d / L)

        cov = rbf_np(t_in, t_in) + np.eye(C) * SIGMA
        km_T = np.linalg.solve(cov, rbf_np(t_in, t_pred))
    return np.ascontiguousarray(km_T.astype(np.float32))


K_MULT_T = _k_mult_T()


def _split16(x: np.ndarray):
    hi = x.astype(np.float16)
    lo = (x - hi.astype(np.float32)).astype(np.float16)
    return hi, lo


def _km_cat() -> np.ndarray:
    """[C, 2*P] fp16: row c = [kh[c, :], kl[c, :]]."""
    kh, kl = _split16(K_MULT_T)
    cat = np.empty((C, 2, P), np.float16)
    cat[:, 0] = kh
    cat[:, 1] = kl
    return np.ascontiguousarray(cat.reshape(C, 2 * P))


KM_CAT = _km_cat()

_NC = None


def _build():
    nc = bacc.Bacc()
    # z rows interleave hi/lo: row r = [zh[r, :], zl[r, :]] (2*D fp16 = 4KB)
    z = nc.dram_tensor("z", [BPC * C, 2 * D], mybir.dt.float16, kind="ExternalInput")
    km = nc.dram_tensor("km", [C, 2 * P], mybir.dt.float16, kind="ExternalInput")
    out = nc.dram_tensor("out", [BPC * P, D], mybir.dt.float32, kind="ExternalOutput")

    with tile.TileContext(nc) as tc, ExitStack() as ctx:
        kpool = ctx.enter_context(tc.tile_pool(name="km", bufs=1))
        zpool = ctx.enter_context(tc.tile_pool(name="z", bufs=12))
        opool = ctx.enter_context(tc.tile_pool(name="o", bufs=3))
        ppool = ctx.enter_context(tc.tile_pool(name="ps", bufs=4, space="PSUM"))

        # k_mult.T staged once: col block j = [kh rows | kl rows] of K-tile j
        km_sb = kpool.tile([128, 4 * 2 * P], mybir.dt.float16)
        for j in range(4):
            nc.sync.dma_start(
                km_sb[: KJ[j], j * 2 * P : (j + 1) * 2 * P],
                km[KOFF[j] : KOFF[j] + KJ[j], :],
            )

        def kh_j(j):
            return km_sb[: KJ[j], j * 2 * P : j * 2 * P + P]

        def kl_j(j):
            return km_sb[: KJ[j], j * 2 * P + P : (j + 1) * 2 * P]

        for bp in range(BPC // 2):  # batch pairs -> [128, D] output tiles
            out_sb = opool.tile([128, D], mybir.dt.float32)
            for half in range(2):
                b = 2 * bp + half
                zt = [
                    zpool.tile([128, 2 * D], mybir.dt.float16,
                               name=f"zt{b}_{j}", tag="zt")
                    for j in range(4)
                ]
                for j in range(4):
                    nc.sync.dma_start(
                        zt[j][: KJ[j], :],
                        z[b * C + KOFF[j] : b * C + KOFF[j] + KJ[j], :],
                    )

                def zh_jn(j, n):
                    return zt[j][: KJ[j], n * 512 : (n + 1) * 512]

                def zl_jn(j, n):
                    return zt[j][: KJ[j], D + n * 512 : D + (n + 1) * 512]

                for n in range(2):  # 512-wide PSUM column halves
                    ps = ppool.tile([P, 512], mybir.dt.float32)
                    terms = (
                        [(kh_j(j), zh_jn(j, n)) for j in range(4)]
                        + [(kh_j(j), zl_jn(j, n)) for j in range(4)]
                        + [(kl_j(j), zh_jn(j, n)) for j in range(4)]
                    )
                    for i, (lhsT, rhs) in enumerate(terms):
                        nc.tensor.matmul(
                            ps[:, :], lhsT, rhs,
                            start=(i == 0), stop=(i == len(terms) - 1),
                        )
                    nc.vector.tensor_copy(
                        out_sb[half * P : (half + 1) * P, n * 512 : (n + 1) * 512],
                        ps[:, :],
                    )
            nc.sync.dma_start(out[bp * 128 : (bp + 1) * 128, :], out_sb[:])

    nc.finalize()
    return nc


def kernel(z_enc: np.ndarray, _trace: bool = False):
    global _NC
    z_enc = np.asarray(z_enc, dtype=np.float32)
    if _NC is None:
        _NC = _build()

    in_maps = []
    for i in range(N_CORES):
        shard = z_enc[i * BPC : (i + 1) * BPC, :C, :].reshape(BPC * C, D)
        hi, lo = _split16(shard)
        cat = np.empty((BPC * C, 2, D), np.float16)
        cat[:, 0] = hi
        cat[:, 1] = lo
        in_maps.append({"z": cat.reshape(BPC * C, 2 * D), "km": KM_CAT})

    res = run_bass_kernel_spmd(_NC, in_maps, core_ids=list(range(N_CORES)),
                               trace=_trace)
    out = np.concatenate(
        [r["out"].reshape(BPC, P, D) for r in res.results], axis=0
    )
    if _trace:
        return out, res
    return out



# revision 3
# speedup vs baseline: 1.8409x; 1.8409x over previous
"""GP prediction kernel for Trainium2 (8 NeuronCores, data-parallel over batch).

Computes z_pred[b, p, d] = sum_c k_mult[p, c] * z_enc[b, c, d] where k_mult
is the [64, 448] GP weight matrix k_pred.T @ inv(cov + sigma*I). k_mult
depends only on compile-time constants, so it is precomputed on host; the
device work is a batched [64,448] @ [448,1024] matmul, sharded 8 batches
per core.

Precision: the correctness gate is rel_err < 2e-2; fp16 operands and fp16
output give ~3.5e-4, so z/k/out all move over HBM as fp16 (half the bytes
of the fp32 baseline). Loads alternate between the two HWDGE queues
(sync=SP, scalar=ACT) and output stores ride the gpsimd SWDGE queue so
DMA bandwidth is not serialized behind a single queue FIFO.
"""
import numpy as np
from contextlib import ExitStack

import concourse.bacc as bacc
import concourse.tile as tile
from concourse import mybir
from concourse.bass_utils import run_bass_kernel_spmd

# Problem constants (hardcoded per harness contract).
B, T, D = 64, 512, 1024
P = 64                 # N_PREDICTORS
C = T - P              # 448 context timesteps
L, SIGMA, TIMESCALE = 0.01, 0.01, 0.3
N_CORES = 8
BPC = B // N_CORES     # batches per core

KJ = [128, 128, 128, 64]          # K-tile sizes along the contraction dim
KOFF = [0, 128, 256, 384]


def _k_mult_T() -> np.ndarray:
    """[C, P] transpose of the GP weight matrix.

    Replicates the reference's fp32 jax ops on CPU so the constant matches
    the reference's k_mult near-bitwise; falls back to a float64 numpy solve.
    """
    try:
        import jax
        import jax.numpy as jnp

        cpu = jax.devices("cpu")[0]
        with jax.default_device(cpu):
            t = jnp.linspace(0.0, 1.0, T)
            t_in = t[:C] * TIMESCALE
            t_pred = t[C:] * TIMESCALE

            def rbf(x, y):
                d = x[:, None] - y[None, :]
                return jnp.exp(-0.5 * (d * d) / L)

            cov = rbf(t_in, t_in)
            k_pred = rbf(t_in, t_pred)
            eye = jnp.eye(C, dtype=cov.dtype)
            k_mult = k_pred.T @ jnp.linalg.inv(cov + eye * SIGMA)   # [P, C]
            km_T = np.asarray(k_mult).T                             # [C, P]
    except Exception:
        t = np.linspace(0.0, 1.0, T)
        t_in = t[:C] * TIMESCALE
        t_pred = t[C:] * TIMESCALE

        def rbf_np(x, y):
            d = x[:, None] - y[None, :]
            return np.exp(-0.5 * d * d / L)

        cov = rbf_np(t_in, t_in) + np.eye(C) * SIGMA
        km_T = np.linalg.solve(cov, rbf_np(t_in, t_pred))
    return np.ascontiguousarray(km_T.astype(np.float32))


def _km_packed() -> np.ndarray:
    """[128, 4*P] fp16: column block j holds K-tile j of k_mult.T (rows
    past KJ[j] zero), so the whole weight matrix arrives in one DMA."""
    km_T = _k_mult_T().astype(np.float16)      # [C, P]
    out = np.zeros((128, 4 * P), np.float16)
    for j in range(4):
        out[: KJ[j], j * P : (j + 1) * P] = km_T[KOFF[j] : KOFF[j] + KJ[j]]
    return np.ascontiguousarray(out)


KM_PACKED = _km_packed()

_NC = None


def _build():
    nc = bacc.Bacc()
    z = nc.dram_tensor("z", [BPC * C, D], mybir.dt.float16, kind="ExternalInput")
    km = nc.dram_tensor("km", [128, 4 * P], mybir.dt.float16, kind="ExternalInput")
    out = nc.dram_tensor("out", [BPC * P, D], mybir.dt.float16, kind="ExternalOutput")

    with tile.TileContext(nc) as tc, ExitStack() as ctx:
        kpool = ctx.enter_context(tc.tile_pool(name="km", bufs=1))
        zpool = ctx.enter_context(tc.tile_pool(name="z", bufs=12))
        opool = ctx.enter_context(tc.tile_pool(name="o", bufs=3))
        ppool = ctx.enter_context(tc.tile_pool(name="ps", bufs=4, space="PSUM"))

        km_sb = kpool.tile([128, 4 * P], mybir.dt.float16)
        nc.scalar.dma_start(km_sb[:, :], km[:, :])

        def km_j(j):
            return km_sb[: KJ[j], j * P : (j + 1) * P]

        for bp in range(BPC // 2):  # batch pairs -> [128, D] fp16 output tiles
            out_sb = opool.tile([128, D], mybir.dt.float16)
            for half in range(2):
                b = 2 * bp + half
                zt = [
                    zpool.tile([128, D], mybir.dt.float16,
                               name=f"zt{b}_{j}", tag="zt")
                    for j in range(4)
                ]
                for j in range(4):
                    eng = nc.sync if (4 * b + j) % 2 == 0 else nc.scalar
                    eng.dma_start(
                        zt[j][: KJ[j], :],
                        z[b * C + KOFF[j] : b * C + KOFF[j] + KJ[j], :],
                    )

                for n in range(2):  # 512-wide PSUM column halves
                    ps = ppool.tile([P, 512], mybir.dt.float32)
                    for j in range(4):
                        nc.tensor.matmul(
                            ps[:, :], km_j(j),
                            zt[j][: KJ[j], n * 512 : (n + 1) * 512],
                            start=(j == 0), stop=(j == 3),
                        )
                    dst = out_sb[half * P : (half + 1) * P, n * 512 : (n + 1) * 512]
                    if (2 * b + n) % 2 == 0:
                        nc.vector.tensor_copy(dst, ps[:, :])
                    else:
                        nc.scalar.activation(
                            dst, ps[:, :], mybir.ActivationFunctionType.Copy
                        )
            nc.gpsimd.dma_start(out[bp * 128 : (bp + 1) * 128, :], out_sb[:])

    nc.finalize()
    return nc


def kernel(z_enc: np.ndarray, _trace: bool = False):
    global _NC
    z_enc = np.asarray(z_enc, dtype=np.float32)
    if _NC is None:
        _NC = _build()

    z16 = z_enc[:, :C, :].astype(np.float16)
    in_maps = []
    for i in range(N_CORES):
        shard = z16[i * BPC : (i + 1) * BPC].reshape(BPC * C, D)
        in_maps.append({"z": shard, "km": KM_PACKED})

    res = run_bass_kernel_spmd(_NC, in_maps, core_ids=list(range(N_CORES)),
                               trace=_trace)
    out = np.concatenate(
        [r["out"].astype(np.float32).reshape(BPC, P, D) for r in res.results],
        axis=0,
    )
    if _trace:
        return out, res
    return out


# revision 4
# speedup vs baseline: 1.8977x; 1.0309x over previous
"""GP prediction kernel for Trainium2 (8 NeuronCores, data-parallel over batch).

Computes z_pred[b, p, d] = sum_c k_mult[p, c] * z_enc[b, c, d] where k_mult
is the [64, 448] GP weight matrix k_pred.T @ inv(cov + sigma*I). k_mult
depends only on compile-time constants, so it is precomputed on host; the
device work is a batched [64,448] @ [448,1024] matmul, sharded 8 batches
per core.

Precision: the correctness gate is rel_err < 2e-2; fp16 operands and fp16
output give ~3.5e-4, so z/k/out all move over HBM as fp16 (half the bytes
of the fp32 baseline). Loads alternate between the two HWDGE queues
(sync=SP, scalar=ACT) and output stores ride the gpsimd SWDGE queue so
DMA bandwidth is not serialized behind a single queue FIFO.
"""
import numpy as np
from contextlib import ExitStack

import concourse.bacc as bacc
import concourse.tile as tile
from concourse import mybir
from concourse.bass_utils import run_bass_kernel_spmd

# Problem constants (hardcoded per harness contract).
B, T, D = 64, 512, 1024
P = 64                 # N_PREDICTORS
C = T - P              # 448 context timesteps
L, SIGMA, TIMESCALE = 0.01, 0.01, 0.3
N_CORES = 8
BPC = B // N_CORES     # batches per core

KJ = [128, 128, 128, 64]          # K-tile sizes along the contraction dim
KOFF = [0, 128, 256, 384]


def _k_mult_T() -> np.ndarray:
    """[C, P] transpose of the GP weight matrix.

    Replicates the reference's fp32 jax ops on CPU so the constant matches
    the reference's k_mult near-bitwise; falls back to a float64 numpy solve.
    """
    try:
        import jax
        import jax.numpy as jnp

        cpu = jax.devices("cpu")[0]
        with jax.default_device(cpu):
            t = jnp.linspace(0.0, 1.0, T)
            t_in = t[:C] * TIMESCALE
            t_pred = t[C:] * TIMESCALE

            def rbf(x, y):
                d = x[:, None] - y[None, :]
                return jnp.exp(-0.5 * (d * d) / L)

            cov = rbf(t_in, t_in)
            k_pred = rbf(t_in, t_pred)
            eye = jnp.eye(C, dtype=cov.dtype)
            k_mult = k_pred.T @ jnp.linalg.inv(cov + eye * SIGMA)   # [P, C]
            km_T = np.asarray(k_mult).T                             # [C, P]
    except Exception:
        t = np.linspace(0.0, 1.0, T)
        t_in = t[:C] * TIMESCALE
        t_pred = t[C:] * TIMESCALE

        def rbf_np(x, y):
            d = x[:, None] - y[None, :]
            return np.exp(-0.5 * d * d / L)

        cov = rbf_np(t_in, t_in) + np.eye(C) * SIGMA
        km_T = np.linalg.solve(cov, rbf_np(t_in, t_pred))
    return np.ascontiguousarray(km_T.astype(np.float32))


def _km_packed() -> np.ndarray:
    """[128, 4*P] fp16: column block j holds K-tile j of k_mult.T (rows
    past KJ[j] zero), so the whole weight matrix arrives in one DMA."""
    km_T = _k_mult_T().astype(np.float16)      # [C, P]
    out = np.zeros((128, 4 * P), np.float16)
    for j in range(4):
        out[: KJ[j], j * P : (j + 1) * P] = km_T[KOFF[j] : KOFF[j] + KJ[j]]
    return np.ascontiguousarray(out)


KM_PACKED = _km_packed()

_NC = None


def _build():
    nc = bacc.Bacc()
    z = nc.dram_tensor("z", [BPC * C, D], mybir.dt.float16, kind="ExternalInput")
    km = nc.dram_tensor("km", [128, 4 * P], mybir.dt.float16, kind="ExternalInput")
    out = nc.dram_tensor("out", [BPC * P, D], mybir.dt.float16, kind="ExternalOutput")

    with tile.TileContext(nc) as tc, ExitStack() as ctx:
        kpool = ctx.enter_context(tc.tile_pool(name="km", bufs=1))
        zpool = ctx.enter_context(tc.tile_pool(name="z", bufs=24))
        opool = ctx.enter_context(tc.tile_pool(name="o", bufs=4))
        ppool = ctx.enter_context(tc.tile_pool(name="ps", bufs=4, space="PSUM"))

        km_sb = kpool.tile([128, 4 * P], mybir.dt.float16)
        nc.scalar.dma_start(km_sb[:, :], km[:, :])

        def km_j(j):
            return km_sb[: KJ[j], j * P : (j + 1) * P]

        for bp in range(BPC // 2):  # batch pairs -> [128, D] fp16 output tiles
            out_sb = opool.tile([128, D], mybir.dt.float16)
            # both batches of the pair: z K-tiles, loads split over the two
            # HWDGE queues (sync=SP, scalar=ACT)
            zt = {}
            for half in range(2):
                b = 2 * bp + half
                zt[half] = [
                    zpool.tile([128, D], mybir.dt.float16,
                               name=f"zt{b}_{j}", tag="zt")
                    for j in range(4)
                ]
                for j in range(4):
                    eng = nc.sync if (4 * b + j) % 2 == 0 else nc.scalar
                    eng.dma_start(
                        zt[half][j][: KJ[j], :],
                        z[b * C + KOFF[j] : b * C + KOFF[j] + KJ[j], :],
                    )

            for n in range(2):  # 512-wide PSUM column halves
                # column-tiled pair: batch 2bp in PE cols 0-63 -> psum rows
                # 0-63, batch 2bp+1 in PE cols 64-127 -> psum rows 64-127;
                # interleaved A/B matmuls run concurrently on the array.
                ps = ppool.tile([128, 512], mybir.dt.float32)
                for j in range(4):
                    for half in range(2):
                        nc.tensor.matmul(
                            ps[half * P : (half + 1) * P, :],
                            km_j(j),
                            zt[half][j][: KJ[j], n * 512 : (n + 1) * 512],
                            start=(j == 0), stop=(j == 3),
                            tile_position=(0, half * P),
                        )
                dst = out_sb[:, n * 512 : (n + 1) * 512]
                if n == 0:
                    nc.vector.tensor_copy(dst, ps[:, :])
                else:
                    nc.scalar.activation(
                        dst, ps[:, :], mybir.ActivationFunctionType.Copy
                    )
            nc.gpsimd.dma_start(out[bp * 128 : (bp + 1) * 128, :], out_sb[:])

    nc.finalize()
    return nc


def kernel(z_enc: np.ndarray, _trace: bool = False):
    global _NC
    z_enc = np.asarray(z_enc, dtype=np.float32)
    if _NC is None:
        _NC = _build()

    z16 = z_enc[:, :C, :].astype(np.float16)
    in_maps = []
    for i in range(N_CORES):
        shard = z16[i * BPC : (i + 1) * BPC].reshape(BPC * C, D)
        in_maps.append({"z": shard, "km": KM_PACKED})

    res = run_bass_kernel_spmd(_NC, in_maps, core_ids=list(range(N_CORES)),
                               trace=_trace)
    out = np.concatenate(
        [r["out"].astype(np.float32).reshape(BPC, P, D) for r in res.results],
        axis=0,
    )
    if _trace:
        return out, res
    return out


# revision 7
# speedup vs baseline: 1.9645x; 1.0352x over previous
"""GP prediction kernel for Trainium2 (8 NeuronCores, data-parallel over batch).

Computes z_pred[b, p, d] = sum_c k_mult[p, c] * z_enc[b, c, d] where k_mult
is the [64, 448] GP weight matrix k_pred.T @ inv(cov + sigma*I). k_mult
depends only on compile-time constants, so it is precomputed on host; the
device work is a batched [64,448] @ [448,1024] matmul, sharded 8 batches
per core.

Precision: the correctness gate is rel_err < 2e-2; fp16 operands and fp16
output give ~3.5e-4, so z/k/out all move over HBM as fp16 (half the bytes
of the fp32 baseline). Loads alternate between the two HWDGE queues
(sync=SP, scalar=ACT) and output stores ride the gpsimd SWDGE queue so
DMA bandwidth is not serialized behind a single queue FIFO.
"""
import numpy as np
from contextlib import ExitStack

import concourse.bacc as bacc
import concourse.tile as tile
from concourse import mybir
from concourse.bass_utils import run_bass_kernel_spmd

# Problem constants (hardcoded per harness contract).
B, T, D = 64, 512, 1024
P = 64                 # N_PREDICTORS
C = T - P              # 448 context timesteps
L, SIGMA, TIMESCALE = 0.01, 0.01, 0.3
N_CORES = 8
BPC = B // N_CORES     # batches per core

KJ = [128, 128, 128, 64]          # K-tile sizes along the contraction dim
KOFF = [0, 128, 256, 384]


def _k_mult_T() -> np.ndarray:
    """[C, P] transpose of the GP weight matrix.

    Replicates the reference's fp32 jax ops on CPU so the constant matches
    the reference's k_mult near-bitwise; falls back to a float64 numpy solve.
    """
    try:
        import jax
        import jax.numpy as jnp

        cpu = jax.devices("cpu")[0]
        with jax.default_device(cpu):
            t = jnp.linspace(0.0, 1.0, T)
            t_in = t[:C] * TIMESCALE
            t_pred = t[C:] * TIMESCALE

            def rbf(x, y):
                d = x[:, None] - y[None, :]
                return jnp.exp(-0.5 * (d * d) / L)

            cov = rbf(t_in, t_in)
            k_pred = rbf(t_in, t_pred)
            eye = jnp.eye(C, dtype=cov.dtype)
            k_mult = k_pred.T @ jnp.linalg.inv(cov + eye * SIGMA)   # [P, C]
            km_T = np.asarray(k_mult).T                             # [C, P]
    except Exception:
        t = np.linspace(0.0, 1.0, T)
        t_in = t[:C] * TIMESCALE
        t_pred = t[C:] * TIMESCALE

        def rbf_np(x, y):
            d = x[:, None] - y[None, :]
            return np.exp(-0.5 * d * d / L)

        cov = rbf_np(t_in, t_in) + np.eye(C) * SIGMA
        km_T = np.linalg.solve(cov, rbf_np(t_in, t_pred))
    return np.ascontiguousarray(km_T.astype(np.float32))


def _km_packed() -> np.ndarray:
    """[128, 4*P] fp16: column block j holds K-tile j of k_mult.T (rows
    past KJ[j] zero), so the whole weight matrix arrives in one DMA."""
    km_T = _k_mult_T().astype(np.float16)      # [C, P]
    out = np.zeros((128, 4 * P), np.float16)
    for j in range(4):
        out[: KJ[j], j * P : (j + 1) * P] = km_T[KOFF[j] : KOFF[j] + KJ[j]]
    return np.ascontiguousarray(out)


KM_PACKED = _km_packed()

_NC = None


def _build():
    nc = bacc.Bacc()
    z = nc.dram_tensor("z", [BPC * C, D], mybir.dt.float16, kind="ExternalInput")
    km = nc.dram_tensor("km", [128, 4 * P], mybir.dt.float16, kind="ExternalInput")
    out = nc.dram_tensor("out", [BPC * P, D], mybir.dt.float16, kind="ExternalOutput")

    with tile.TileContext(nc) as tc, ExitStack() as ctx:
        kpool = ctx.enter_context(tc.tile_pool(name="km", bufs=1))
        zpool = ctx.enter_context(tc.tile_pool(name="z", bufs=32))
        opool = ctx.enter_context(tc.tile_pool(name="o", bufs=4))
        ppool = ctx.enter_context(tc.tile_pool(name="ps", bufs=4, space="PSUM"))

        km_sb = kpool.tile([128, 4 * P], mybir.dt.float16)
        nc.scalar.dma_start(km_sb[:, :], km[:, :])

        def km_j(j):
            return km_sb[: KJ[j], j * P : (j + 1) * P]

        for bp in range(BPC // 2):  # batch pairs -> [128, D] fp16 output tiles
            out_sb = opool.tile([128, D], mybir.dt.float16)
            # both batches of the pair: z K-tiles, loads split over the two
            # HWDGE queues (sync=SP, scalar=ACT)
            zt = {}
            for half in range(2):
                b = 2 * bp + half
                zt[half] = [
                    zpool.tile([128, D], mybir.dt.float16,
                               name=f"zt{b}_{j}", tag="zt")
                    for j in range(4)
                ]
                for j in range(4):
                    eng = nc.sync if (4 * b + j) % 2 == 0 else nc.scalar
                    eng.dma_start(
                        zt[half][j][: KJ[j], :],
                        z[b * C + KOFF[j] : b * C + KOFF[j] + KJ[j], :],
                    )

            # column-tiled pair: batch 2bp in PE cols 0-63 -> psum rows
            # 0-63, batch 2bp+1 in PE cols 64-127 -> psum rows 64-127;
            # interleaved A/B matmuls run concurrently on the array.
            # j outermost so one weight tile serves 4 consecutive matmuls.
            pss = [
                ppool.tile([128, 512], mybir.dt.float32, name=f"ps{bp}_{n}",
                           tag="ps")
                for n in range(2)
            ]
            for j in range(4):
                for n in range(2):
                    for half in range(2):
                        nc.tensor.matmul(
                            pss[n][half * P : (half + 1) * P, :],
                            km_j(j),
                            zt[half][j][: KJ[j], n * 512 : (n + 1) * 512],
                            start=(j == 0), stop=(j == 3),
                            tile_position=(0, half * P),
                        )
            for n in range(2):
                dst = out_sb[:, n * 512 : (n + 1) * 512]
                if n == 0:
                    nc.vector.tensor_copy(dst, pss[n][:, :])
                else:
                    nc.scalar.activation(
                        dst, pss[n][:, :], mybir.ActivationFunctionType.Copy
                    )
            oeng = nc.sync if bp % 2 == 0 else nc.scalar
            oeng.dma_start(out[bp * 128 : (bp + 1) * 128, :], out_sb[:])

    nc.finalize()
    return nc


def kernel(z_enc: np.ndarray, _trace: bool = False):
    global _NC
    z_enc = np.asarray(z_enc, dtype=np.float32)
    if _NC is None:
        _NC = _build()

    z16 = z_enc[:, :C, :].astype(np.float16)
    in_maps = []
    for i in range(N_CORES):
        shard = z16[i * BPC : (i + 1) * BPC].reshape(BPC * C, D)
        in_maps.append({"z": shard, "km": KM_PACKED})

    res = run_bass_kernel_spmd(_NC, in_maps, core_ids=list(range(N_CORES)),
                               trace=_trace)
    out = np.concatenate(
        [r["out"].astype(np.float32).reshape(BPC, P, D) for r in res.results],
        axis=0,
    )
    if _trace:
        return out, res
    return out


# revision 8
# speedup vs baseline: 1.9656x; 1.0006x over previous
"""GP prediction kernel for Trainium2 (8 NeuronCores, data-parallel over batch).

Computes z_pred[b, p, d] = sum_c k_mult[p, c] * z_enc[b, c, d] where k_mult
is the [64, 448] GP weight matrix k_pred.T @ inv(cov + sigma*I). k_mult
depends only on compile-time constants, so it is precomputed on host; the
device work is a batched [64,448] @ [448,1024] matmul, sharded 8 batches
per core.

Precision: the correctness gate is rel_err < 2e-2; fp16 operands and fp16
output give ~3.5e-4, so z/k/out all move over HBM as fp16 (half the bytes
of the fp32 baseline). Loads alternate between the two HWDGE queues
(sync=SP, scalar=ACT) and output stores ride the gpsimd SWDGE queue so
DMA bandwidth is not serialized behind a single queue FIFO.
"""
import numpy as np
from contextlib import ExitStack

import concourse.bacc as bacc
import concourse.tile as tile
from concourse import mybir
from concourse.bass_utils import run_bass_kernel_spmd

# Problem constants (hardcoded per harness contract).
B, T, D = 64, 512, 1024
P = 64                 # N_PREDICTORS
C = T - P              # 448 context timesteps
L, SIGMA, TIMESCALE = 0.01, 0.01, 0.3
N_CORES = 8
BPC = B // N_CORES     # batches per core

KJ = [128, 128, 128, 64]          # K-tile sizes along the contraction dim
KOFF = [0, 128, 256, 384]


def _k_mult_T() -> np.ndarray:
    """[C, P] transpose of the GP weight matrix.

    Replicates the reference's fp32 jax ops on CPU so the constant matches
    the reference's k_mult near-bitwise; falls back to a float64 numpy solve.
    """
    try:
        import jax
        import jax.numpy as jnp

        cpu = jax.devices("cpu")[0]
        with jax.default_device(cpu):
            t = jnp.linspace(0.0, 1.0, T)
            t_in = t[:C] * TIMESCALE
            t_pred = t[C:] * TIMESCALE

            def rbf(x, y):
                d = x[:, None] - y[None, :]
                return jnp.exp(-0.5 * (d * d) / L)

            cov = rbf(t_in, t_in)
            k_pred = rbf(t_in, t_pred)
            eye = jnp.eye(C, dtype=cov.dtype)
            k_mult = k_pred.T @ jnp.linalg.inv(cov + eye * SIGMA)   # [P, C]
            km_T = np.asarray(k_mult).T                             # [C, P]
    except Exception:
        t = np.linspace(0.0, 1.0, T)
        t_in = t[:C] * TIMESCALE
        t_pred = t[C:] * TIMESCALE

        def rbf_np(x, y):
            d = x[:, None] - y[None, :]
            return np.exp(-0.5 * d * d / L)

        cov = rbf_np(t_in, t_in) + np.eye(C) * SIGMA
        km_T = np.linalg.solve(cov, rbf_np(t_in, t_pred))
    return np.ascontiguousarray(km_T.astype(np.float32))


def _km_packed() -> np.ndarray:
    """[128, 4*P] fp16: column block j holds K-tile j of k_mult.T (rows
    past KJ[j] zero), so the whole weight matrix arrives in one DMA."""
    km_T = _k_mult_T().astype(np.float16)      # [C, P]
    out = np.zeros((128, 4 * P), np.float16)
    for j in range(4):
        out[: KJ[j], j * P : (j + 1) * P] = km_T[KOFF[j] : KOFF[j] + KJ[j]]
    return np.ascontiguousarray(out)


KM_PACKED = _km_packed()

_NC = None


def _build():
    nc = bacc.Bacc()
    z = nc.dram_tensor("z", [BPC * C, D], mybir.dt.float16, kind="ExternalInput")
    km = nc.dram_tensor("km", [128, 4 * P], mybir.dt.float16, kind="ExternalInput")
    out = nc.dram_tensor("out", [BPC * P, D], mybir.dt.float16, kind="ExternalOutput")

    with tile.TileContext(nc) as tc, ExitStack() as ctx:
        kpool = ctx.enter_context(tc.tile_pool(name="km", bufs=1))
        zpool = ctx.enter_context(tc.tile_pool(name="z", bufs=32))
        opool = ctx.enter_context(tc.tile_pool(name="o", bufs=4))
        ppool = ctx.enter_context(tc.tile_pool(name="ps", bufs=4, space="PSUM"))

        km_sb = kpool.tile([128, 4 * P], mybir.dt.float16)
        nc.scalar.dma_start(km_sb[:, :], km[:, :])

        def km_j(j):
            return km_sb[: KJ[j], j * P : (j + 1) * P]

        # Phase 1: issue ALL z loads up front (32 tiles = whole shard fits in
        # SBUF), split over the two HWDGE queues. Nothing ever blocks these,
        # so both queues stream flat out; compute chases the loads.
        zt = {}
        for b in range(BPC):
            zt[b] = [
                zpool.tile([128, D], mybir.dt.float16, name=f"zt{b}_{j}",
                           tag="zt")
                for j in range(4)
            ]
            for j in range(4):
                eng = nc.sync if (4 * b + j) % 2 == 0 else nc.scalar
                eng.dma_start(
                    zt[b][j][: KJ[j], :],
                    z[b * C + KOFF[j] : b * C + KOFF[j] + KJ[j], :],
                )

        # Phase 2: column-tiled batch pairs: batch 2bp in PE cols 0-63 ->
        # psum rows 0-63, batch 2bp+1 in cols 64-127 -> psum rows 64-127;
        # interleaved A/B matmuls run concurrently on the array. j outermost
        # so one weight tile serves 4 consecutive matmuls.
        for bp in range(BPC // 2):
            out_sb = opool.tile([128, D], mybir.dt.float16, name=f"osb{bp}",
                                tag="osb")
            pss = [
                ppool.tile([128, 512], mybir.dt.float32, name=f"ps{bp}_{n}",
                           tag="ps")
                for n in range(2)
            ]
            for j in range(4):
                for n in range(2):
                    for half in range(2):
                        nc.tensor.matmul(
                            pss[n][half * P : (half + 1) * P, :],
                            km_j(j),
                            zt[2 * bp + half][j][: KJ[j],
                                                 n * 512 : (n + 1) * 512],
                            start=(j == 0), stop=(j == 3),
                            tile_position=(0, half * P),
                        )
            for n in range(2):
                nc.vector.tensor_copy(
                    out_sb[:, n * 512 : (n + 1) * 512], pss[n][:, :]
                )
            oeng = nc.sync if bp % 2 == 0 else nc.scalar
            oeng.dma_start(out[bp * 128 : (bp + 1) * 128, :], out_sb[:])

    nc.finalize()
    return nc


def kernel(z_enc: np.ndarray, _trace: bool = False):
    global _NC
    z_enc = np.asarray(z_enc, dtype=np.float32)
    if _NC is None:
        _NC = _build()

    z16 = z_enc[:, :C, :].astype(np.float16)
    in_maps = []
    for i in range(N_CORES):
        shard = z16[i * BPC : (i + 1) * BPC].reshape(BPC * C, D)
        in_maps.append({"z": shard, "km": KM_PACKED})

    res = run_bass_kernel_spmd(_NC, in_maps, core_ids=list(range(N_CORES)),
                               trace=_trace)
    out = np.concatenate(
        [r["out"].astype(np.float32).reshape(BPC, P, D) for r in res.results],
        axis=0,
    )
    if _trace:
        return out, res
    return out
